# revision 23
# baseline (speedup 1.0000x reference)
"""BroadcastAttention Trainium2 kernel (8 NeuronCores, data-parallel over batch).

Math per sample (C=512, N=4096, H=8 heads, HD=64):
    qkv = Wqkv @ x            # [H*(1+2HD), N]
    q[h,n], k[h,d,n], v[h,d,n] split per head
    s = softmax(q over n)     # [H, N]
    ctx[h,d] = sum_n k[h,d,n]*s[h,n]
    out = Wp @ (relu(v)*ctx) + bp

Key restructure vs the straightforward formulation: the full k-projection
GEMM (Wk @ x, 512x512x4096) is algebraically eliminated:
    ctx[h,:] = Wk_h @ (x @ s_h) / Z_h
so only xs[h,c] = sum_n exp(q[h,n]) * x[c,n] is needed -- a contraction
over n. x^T tiles for that contraction come from the XBAR DMA transpose
(dma_start_transpose, 14ns per 16x128 tile, entirely off the PE).
Additionally relu(v)*ctx is folded into the output projection weights
(Wp'[o,c] = Wp[o,c]*ctxvec[c]), so the V phase never waits on ctx and its
PSUM eviction is a pure relu.

Per-core PE work per sample is then: v-GEMM 27us + p-GEMM 27us +
qT-GEMM 7us + xs-matmuls 7.4us + ctx finalize ~1us (vs ~93us before).

Other implementation notes:
    - x loads via gpsimd cast-DMA (fp32 HBM -> bf16 SBUF directly): no
      staging buffers, no ScalarE/DVE conversion work.
    - q computed as qT[16(pad),N] with wq stationary (cheap LDWEIGHTS);
      exp fused with Z accumulation via activation(accum_out); exp'd
      scores transposed to [n,16] via XBAR for the xs matmul stationary.
    - PSUM evictions (relu-v, p-copy) alternate DVE/ScalarE.
    - Emission order pipelines the two samples per core: b1's V/q chunks
      are emitted around b0's P-GEMM to hide the ctx-finalize latency.
"""

import sys

for _p in ("/opt/trn_rl_repo",):
    if _p not in sys.path:
        sys.path.insert(0, _p)

from contextlib import ExitStack

import ml_dtypes
import numpy as np

import concourse.bass as bass
import concourse.mybir as mybir
import concourse.tile as tile
from concourse import bacc
from concourse.bass_utils import run_bass_kernel_spmd
from concourse.masks import make_identity

# Problem constants (hardcoded per contract; kernel.py must be self-contained).
B, C, N = 16, 512, 4096
H, HD = 8, 64
H16 = 16           # q head dim padded to XBAR row-tile granularity
NCORES = 8
BPC = B // NCORES  # samples per core
CT = C // 128      # 4 contraction/partition tiles of 128
NT = N // 128      # 32 n-tiles
FREE = 512         # matmul moving free-dim chunk
NCH = N // FREE    # 8 chunks
HALF = N // 2      # x loads in half-sample tiles
FP = mybir.dt.float32
BF = mybir.dt.bfloat16  # matmul operand dtype (fp32 PSUM accumulation)

# Results of the last run (for test harness introspection).
LAST_RESULTS = None


def _build(has_qkv_bias: bool, has_p_bias: bool) -> bass.Bass:
    nc = bacc.Bacc("TRN2", target_bir_lowering=False, debug=False)

    x_d = nc.declare_dram_parameter("x", [BPC, C, N], BF, isOutput=False)
    wk_d = nc.declare_dram_parameter("wkT", [C, C], BF, isOutput=False)
    wv_d = nc.declare_dram_parameter("wvT", [C, C], BF, isOutput=False)
    wp_d = nc.declare_dram_parameter("wpT", [C, C], BF, isOutput=False)
    wq_d = nc.declare_dram_parameter("wqT", [C, H16], BF, isOutput=False)
    mask_d = nc.declare_dram_parameter("maskhd", [C, H], FP, isOutput=False)
    bq_d = nc.declare_dram_parameter("bq", [1, H16], BF, isOutput=False)
    bk_d = nc.declare_dram_parameter("bk16", [H16, C], FP, isOutput=False)
    bv_d = nc.declare_dram_parameter("bv", [1, C], BF, isOutput=False)
    bp_d = nc.declare_dram_parameter("bp", [C], FP, isOutput=False)
    y_d = nc.declare_dram_parameter("y", [BPC, C, N], FP, isOutput=True)

    AF = mybir.ActivationFunctionType
    OP = mybir.AluOpType

    with tile.TileContext(nc) as tc, ExitStack() as ctx:
        consts = ctx.enter_context(tc.tile_pool(name="consts", bufs=1))
        xpool = ctx.enter_context(tc.tile_pool(name="xpool", bufs=3))
        xtpool = ctx.enter_context(tc.tile_pool(name="xtpool", bufs=1))
        apool = ctx.enter_context(tc.tile_pool(name="apool", bufs=2))
        qpool = ctx.enter_context(tc.tile_pool(name="qpool", bufs=2))
        spool = ctx.enter_context(tc.tile_pool(name="spool", bufs=2))
        small = ctx.enter_context(tc.tile_pool(name="small", bufs=2))
        wppool = ctx.enter_context(tc.tile_pool(name="wppool", bufs=2))
        opool = ctx.enter_context(tc.tile_pool(name="opool", bufs=4))
        ps_mm = ctx.enter_context(tc.tile_pool(name="ps_mm", bufs=3, space="PSUM"))
        ps_q = ctx.enter_context(tc.tile_pool(name="ps_q", bufs=3, space="PSUM"))
        ps_ctx = ctx.enter_context(tc.tile_pool(name="ps_ctx", bufs=1, space="PSUM"))

        # ---- weights into SBUF (single rearranged DMAs, on Sync/HWDGE) ----
        wk_sb = consts.tile([128, CT, C], BF)
        wv_sb = consts.tile([128, CT, C], BF)
        wp_sb = consts.tile([128, CT, C], BF)
        wq_sb = consts.tile([128, CT, H16], BF)
        mask_sb = consts.tile([128, CT, H], FP)
        bp_sb = consts.tile([128, CT], FP)
        # Weights issue on ScalarE's HWDGE queue so the Sync queue is free
        # for x loads from the first instruction. wv (needed by the first
        # V matmuls) goes first as 4 contiguous per-ct DMAs (fast), the
        # rest as single rearranged DMAs.
        for ct in range(CT):
            nc.scalar.dma_start(
                out=wv_sb[:, ct, :], in_=wv_d[ct * 128:(ct + 1) * 128, :]
            )
        nc.scalar.dma_start(
            out=wq_sb[:], in_=wq_d.rearrange("(ct p) h -> p ct h", p=128)
        )
        nc.scalar.dma_start(
            out=wk_sb[:], in_=wk_d.rearrange("(ct p) o -> p ct o", p=128)
        )
        nc.scalar.dma_start(
            out=wp_sb[:], in_=wp_d.rearrange("(ct p) o -> p ct o", p=128)
        )
        nc.scalar.dma_start(
            out=mask_sb[:], in_=mask_d.rearrange("(ct p) h -> p ct h", p=128)
        )
        nc.scalar.dma_start(out=bp_sb[:], in_=bp_d.rearrange("(o p) -> p o", p=128))

        identity = consts.tile([128, 128], FP)
        make_identity(nc, identity[:])

        if has_qkv_bias:
            bq_sb = consts.tile([1, H16], BF)
            bk_sb = consts.tile([H16, C], FP)
            bv_sb = consts.tile([1, C], BF)
            ones_row = consts.tile([1, FREE], BF)
            nc.sync.dma_start(out=bq_sb[:], in_=bq_d[:, :])
            nc.sync.dma_start(out=bk_sb[:], in_=bk_d[:, :])
            nc.sync.dma_start(out=bv_sb[:], in_=bv_d[:, :])
            nc.vector.memset(ones_row[:], 1.0)

        # Per-sample state carried between emission phases.
        st = [dict() for _ in range(BPC)]

        def emit_x_load_half(b, half, eng, widths):
            """One half-sample of x (bf16, host-converted) HBM -> SBUF."""
            xh = xpool.tile([128, CT, HALF], BF, tag="xh", name="xh")
            pos = 0
            for w in widths:
                for ct in range(CT):
                    eng.dma_start(
                        out=xh[:, ct, pos:pos + w],
                        in_=x_d[b, ct * 128:(ct + 1) * 128,
                                half * HALF + pos:half * HALF + pos + w],
                    )
                pos += w
            st[b].setdefault("xh", {})[half] = xh

        def emit_xT(b):
            """XBAR transposes x^T tiles: per (ct, half) one instruction."""
            xT = xtpool.tile([128, NT, CT, 128], BF, tag="xT", name="xT")
            for half in range(2):
                xh = st[b]["xh"][half]
                for ct in range(CT):
                    nc.sync.dma_start_transpose(
                        out=xT[:, half * (NT // 2):(half + 1) * (NT // 2), ct, :],
                        in_=xh[:, ct, :],
                    )
            st[b]["xT"] = xT

        def emit_vq(b, chunks):
            """V-GEMM blocks + qT-GEMM + evictions for the given chunks."""
            if "a" not in st[b]:
                st[b]["a"] = apool.tile([128, CT, N], BF, tag="a", name="a_sb")
                st[b]["expq"] = qpool.tile([H16, N], BF, tag="expq", name="expq")
                st[b]["zc"] = small.tile([H16, NCH], FP, tag="zc", name="zc")
            a_sb = st[b]["a"]
            expq = st[b]["expq"]
            zc = st[b]["zc"]
            for chk in chunks:
                half, lsl = chk // (NCH // 2), chk % (NCH // 2)
                xh = st[b]["xh"][half]
                csl = slice(chk * FREE, (chk + 1) * FREE)
                lcsl = slice(lsl * FREE, (lsl + 1) * FREE)
                # q first: its exp gates the scoresT transpose -> xs phase,
                # so get it off the PE before the chunk's V matmuls.
                q_ps = ps_q.tile(
                    [H16, FREE], FP, tag="q2k", name="q_ps",
                    padded_shape=[128, FREE],
                )
                for ct in range(CT):
                    last = (ct == CT - 1) and not has_qkv_bias
                    nc.tensor.matmul(
                        q_ps[:], wq_sb[:, ct, :], xh[:, ct, lcsl],
                        start=(ct == 0), stop=last,
                    )
                if has_qkv_bias:
                    nc.tensor.matmul(
                        q_ps[:], bq_sb[:], ones_row[:], start=False, stop=True
                    )
                # exp + per-chunk Z accumulation in one ScalarE op.
                nc.scalar.activation(
                    out=expq[:, csl], in_=q_ps[:], func=AF.Exp,
                    accum_out=zc[:, chk:chk + 1],
                )
                for i in range(CT):
                    v_ps = ps_mm.tile([128, FREE], FP, tag="mm512", name="v_ps")
                    for ct in range(CT):
                        last = (ct == CT - 1) and not has_qkv_bias
                        nc.tensor.matmul(
                            v_ps[:],
                            wv_sb[:, ct, i * 128:(i + 1) * 128],
                            xh[:, ct, lcsl],
                            start=(ct == 0), stop=last,
                        )
                    if has_qkv_bias:
                        nc.tensor.matmul(
                            v_ps[:], bv_sb[:, i * 128:(i + 1) * 128],
                            ones_row[:], start=False, stop=True,
                        )
                    # Pure relu eviction (ctx folded into Wp'):
                    # alternate ScalarE/DVE.
                    if (chk + i) % 2 == 0:
                        nc.scalar.activation(
                            out=a_sb[:, i, csl], in_=v_ps[:], func=AF.Relu
                        )
                    else:
                        nc.vector.tensor_scalar_max(
                            out=a_sb[:, i, csl], in0=v_ps[:], scalar1=0.0
                        )
        def emit_scoresT(b):
            """exp'd scores to [n,16] orientation via XBAR. All XBAR
            transposes stay on the Sync queue: the crossbar is a shared
            resource and concurrent transposes from two queues corrupt
            each other."""
            scoresT = spool.tile(
                [128, NT, H16], BF, tag="scoresT", name="scoresT"
            )
            nc.sync.dma_start_transpose(out=scoresT[:], in_=st[b]["expq"][:])
            st[b]["scoresT"] = scoresT

        def emit_xs(b):
            """xs[h,c] = sum_n exp(q)[n,h] x[c,n]."""
            scoresT = st[b]["scoresT"]
            xT = st[b]["xT"]
            xs_ps = ps_ctx.tile(
                [H16, C], FP, tag="ctx2k", name="xs_ps", padded_shape=[128, C]
            )
            for nt in range(NT):
                nc.tensor.matmul(
                    xs_ps[:], scoresT[:, nt, :], xT[:, nt, :, :],
                    start=(nt == 0), stop=(nt == NT - 1),
                )
            st[b]["xs_ps"] = xs_ps

        def emit_fin(b):
            """ctx = (xs @ WkT)/Z -> ctxv[c] -> Wp' = Wp * ctxv."""
            xs_ps = st[b]["xs_ps"]
            xs_sb = small.tile([H16, C], BF, tag="xs_sb", name="xs_sb")
            nc.vector.tensor_copy(out=xs_sb[:], in_=xs_ps[:])
            xsT = small.tile([128, CT, H16], BF, tag="xsT", name="xsT")
            nc.sync.dma_start_transpose(out=xsT[:], in_=xs_sb[:])
            ctxf_ps = ps_q.tile(
                [H16, C], FP, tag="q2k", name="ctxf_ps", padded_shape=[128, C]
            )
            for ct in range(CT):
                nc.tensor.matmul(
                    ctxf_ps[:], xsT[:, ct, :], wk_sb[:, ct, :],
                    start=(ct == 0), stop=(ct == CT - 1),
                )
            zsum = small.tile([H16, 1], FP, tag="zsum", name="zsum")
            nc.vector.reduce_sum(
                out=zsum[:], in_=st[b]["zc"][:], axis=mybir.AxisListType.X
            )
            invz = small.tile([H16, 1], FP, tag="invz", name="invz")
            nc.vector.reciprocal(out=invz[:], in_=zsum[:])
            ctxn = small.tile([H16, C], FP, tag="ctxn", name="ctxn")
            nc.vector.tensor_scalar_mul(out=ctxn[:], in0=ctxf_ps[:], scalar1=invz[:])
            if has_qkv_bias:
                ctxb = small.tile([H16, C], FP, tag="ctxb", name="ctxb")
                nc.vector.tensor_tensor(
                    out=ctxb[:], in0=ctxn[:], in1=bk_sb[:], op=OP.add
                )
                ctxn = ctxb
            ctxv = small.tile([128, CT], FP, tag="ctxv", name="ctxv")
            for i in range(CT):
                ctxnT_ps = ps_q.tile(
                    [128, H16], FP, tag="q2k", name="ctxnT_ps",
                    padded_shape=[128, C],
                )
                nc.tensor.transpose(
                    ctxnT_ps[:], ctxn[:, i * 128:(i + 1) * 128],
                    identity[:H16, :H16],
                )
                junk = small.tile([128, H], FP, tag="junk", name="junk")
                nc.vector.tensor_tensor(
                    out=junk[:], in0=ctxnT_ps[:, :H], in1=mask_sb[:, i, :],
                    op=OP.mult,
                )
                nc.vector.reduce_sum(
                    out=ctxv[:, i:i + 1], in_=junk[:], axis=mybir.AxisListType.X
                )
            wp2 = wppool.tile([128, CT, C], BF, tag="wp2", name="wp2")
            for ct in range(CT):
                nc.vector.tensor_scalar_mul(
                    out=wp2[:, ct, :], in0=wp_sb[:, ct, :],
                    scalar1=ctxv[:, ct:ct + 1],
                )
            st[b]["wp2"] = wp2

        def emit_p(b, o_range=None):
            """Output projection with folded weights + bias + store."""
            a_sb = st[b]["a"]
            wp2 = st[b]["wp2"]
            QS = 1024  # output staging quarter-rows
            for o in (o_range if o_range is not None else range(CT)):
                for q4 in range(4):
                    o_sb = opool.tile([128, QS], FP, tag="osb", name="o_sb")
                    for lc in range(2):
                        chk = q4 * 2 + lc
                        p_ps = ps_mm.tile([128, FREE], FP, tag="mm512", name="p_ps")
                        csl = slice(chk * FREE, (chk + 1) * FREE)
                        for c2 in range(CT):
                            nc.tensor.matmul(
                                p_ps[:],
                                wp2[:, c2, o * 128:(o + 1) * 128],
                                a_sb[:, c2, csl],
                                start=(c2 == 0), stop=(c2 == CT - 1),
                            )
                        osl = slice(lc * FREE, (lc + 1) * FREE)
                        if has_p_bias:
                            if chk % 2 == 0:
                                nc.vector.tensor_scalar_add(
                                    out=o_sb[:, osl], in0=p_ps[:],
                                    scalar1=bp_sb[:, o:o + 1],
                                )
                            else:
                                nc.scalar.add(
                                    o_sb[:, osl], p_ps[:], add=bp_sb[:, o:o + 1]
                                )
                        else:
                            if chk % 2 == 0:
                                nc.vector.tensor_copy(out=o_sb[:, osl], in_=p_ps[:])
                            else:
                                nc.scalar.copy(o_sb[:, osl], p_ps[:])
                    # y stores alternate between the ScalarE and Sync HWDGE
                    # queues: two queues keep more store DMAs in flight,
                    # which shortens the end-of-kernel drain.
                    eng = nc.scalar if (o * 4 + q4) % 2 == 0 else nc.sync
                    eng.dma_start(
                        out=y_d[b, o * 128:(o + 1) * 128,
                                q4 * QS:(q4 + 1) * QS],
                        in_=o_sb[:],
                    )

        # ---- emission schedule (PE queue order == emission order) ----
        # Queue discipline: all XBAR transposes serialize on Sync (the
        # crossbar is shared; DMA_TRANSPOSE also occupies its sequencer for
        # the whole transfer). x loads that would queue behind transposes go
        # on Scalar instead. b1's h1 load is emitted after b0's x readers so
        # the xpool slot-reuse dependencies cover all of b0's accesses.
        # b1 V/q chunks are emitted around b0's finalize and P phases so the
        # in-order PE queue never waits on the ctx-finalize DMA/DVE chains.
        emit_x_load_half(0, 0, nc.sync, [512, 512, 1024])
        emit_x_load_half(0, 1, nc.sync, [2048])
        emit_x_load_half(1, 0, nc.scalar, [2048])
        emit_vq(0, range(NCH))
        emit_xT(0)
        emit_x_load_half(1, 1, nc.sync, [2048])
        emit_scoresT(0)
        emit_xs(0)
        emit_vq(1, range(0, 2))
        emit_fin(0)
        emit_vq(1, range(2, NCH))
        emit_xT(1)
        emit_scoresT(1)
        emit_p(0, range(0, 3))
        emit_xs(1)
        emit_fin(1)
        emit_p(0, range(3, CT))  # hides b1's ctx-finalize latency
        emit_p(1)

    nc.compile()
    return nc


_NC_CACHE = {}


def kernel(x, Wqkv, bqkv, Wp, bp):
    global LAST_RESULTS
    bf16 = ml_dtypes.bfloat16
    # x is consumed by the device in bf16 (fp32 streams through the PE at
    # half rate); converting on host halves the x HBM traffic as well.
    x = np.ascontiguousarray(np.asarray(x, dtype=np.float32).astype(bf16))
    Wqkv = np.asarray(Wqkv, dtype=np.float32)
    bqkv = np.asarray(bqkv, dtype=np.float32)
    Wp = np.asarray(Wp, dtype=np.float32)
    bp = np.asarray(bp, dtype=np.float32)

    # Host-side weight layout prep (tiny, one-time).
    r = Wqkv.reshape(H, 1 + 2 * HD, C)
    wqT = np.zeros((C, H16), dtype=np.float32)
    wqT[:, :H] = r[:, 0, :].T
    wqT = wqT.astype(bf16)
    wkT = np.ascontiguousarray(r[:, 1:1 + HD, :].reshape(C, C).T).astype(bf16)
    wvT = np.ascontiguousarray(r[:, 1 + HD:, :].reshape(C, C).T).astype(bf16)
    wpT = np.ascontiguousarray(Wp.T).astype(bf16)                     # [C, o]
    rb = bqkv.reshape(H, 1 + 2 * HD)
    bq = np.zeros((1, H16), dtype=np.float32)
    bq[0, :H] = rb[:, 0]
    bq = bq.astype(bf16)
    bk16 = np.tile(
        rb[:, 1:1 + HD].reshape(1, C), (H16, 1)
    ).astype(np.float32)
    bv = np.ascontiguousarray(rb[:, 1 + HD:].reshape(1, C)).astype(bf16)
    maskhd = np.zeros((C, H), dtype=np.float32)
    for ch in range(C):
        maskhd[ch, ch // HD] = 1.0

    has_qkv_bias = bool(np.any(bqkv != 0.0))
    has_p_bias = bool(np.any(bp != 0.0))

    key = (has_qkv_bias, has_p_bias)
    if key not in _NC_CACHE:
        _NC_CACHE[key] = _build(*key)
    nc = _NC_CACHE[key]

    shared = {
        "wkT": wkT, "wvT": wvT, "wpT": wpT, "wqT": wqT,
        "maskhd": maskhd, "bq": bq, "bk16": bk16, "bv": bv, "bp": bp,
    }
    in_maps = [
        {"x": x[i * BPC:(i + 1) * BPC], **shared} for i in range(NCORES)
    ]
    LAST_RESULTS = run_bass_kernel_spmd(nc, in_maps, list(range(NCORES)))
    out = np.concatenate(
        [LAST_RESULTS.results[i]["y"] for i in range(NCORES)], axis=0
    )
    return out.astype(np.float32)


if __name__ == "__main__":
    rng = np.random.default_rng(0)
    x = rng.standard_normal((B, C, N), dtype=np.float32)
    Wqkv = (rng.standard_normal((H * (1 + 2 * HD), C), dtype=np.float32) * 0.02)
    bqkv = np.zeros((H * (1 + 2 * HD),), np.float32)
    Wp = rng.standard_normal((C, C), dtype=np.float32) * 0.02
    bp = np.zeros((C,), np.float32)
    y = kernel(x, Wqkv, bqkv, Wp, bp)
    print("out", y.shape, y.dtype)


# revision 28
# speedup vs baseline: 1.0717x; 1.0717x over previous
"""BroadcastAttention Trainium2 kernel (8 NeuronCores, data-parallel over batch).

Math per sample (C=512, N=4096, H=8 heads, HD=64):
    qkv = Wqkv @ x            # [H*(1+2HD), N]
    q[h,n], k[h,d,n], v[h,d,n] split per head
    s = softmax(q over n)     # [H, N]
    ctx[h,d] = sum_n k[h,d,n]*s[h,n]
    out = Wp @ (relu(v)*ctx) + bp

Key restructure vs the straightforward formulation: the full k-projection
GEMM (Wk @ x, 512x512x4096) is algebraically eliminated:
    ctx[h,:] = Wk_h @ (x @ s_h) / Z_h
so only xs[h,c] = sum_n exp(q[h,n]) * x[c,n] is needed -- a contraction
over n. x^T tiles for that contraction come from the XBAR DMA transpose
(dma_start_transpose, 14ns per 16x128 tile, entirely off the PE).
Additionally relu(v)*ctx is folded into the output projection weights
(Wp'[o,c] = Wp[o,c]*ctxvec[c]), so the V phase never waits on ctx and its
PSUM eviction is a pure relu.

Per-core PE work per sample is then: v-GEMM 27us + p-GEMM 27us +
qT-GEMM 7us + xs-matmuls 7.4us + ctx finalize ~1us (vs ~93us before).

Other implementation notes:
    - x loads via gpsimd cast-DMA (fp32 HBM -> bf16 SBUF directly): no
      staging buffers, no ScalarE/DVE conversion work.
    - q computed as qT[16(pad),N] with wq stationary (cheap LDWEIGHTS);
      exp fused with Z accumulation via activation(accum_out); exp'd
      scores transposed to [n,16] via XBAR for the xs matmul stationary.
    - PSUM evictions (relu-v, p-copy) alternate DVE/ScalarE.
    - Emission order pipelines the two samples per core: b1's V/q chunks
      are emitted around b0's P-GEMM to hide the ctx-finalize latency.
"""

import sys

for _p in ("/opt/trn_rl_repo",):
    if _p not in sys.path:
        sys.path.insert(0, _p)

from contextlib import ExitStack

import ml_dtypes
import numpy as np

import concourse.bass as bass
import concourse.mybir as mybir
import concourse.tile as tile
from concourse import bacc
from concourse.bass_utils import run_bass_kernel_spmd
from concourse.masks import make_identity

# Problem constants (hardcoded per contract; kernel.py must be self-contained).
B, C, N = 16, 512, 4096
H, HD = 8, 64
H16 = 16           # q head dim padded to XBAR row-tile granularity
NCORES = 8
BPC = B // NCORES  # samples per core
CT = C // 128      # 4 contraction/partition tiles of 128
NT = N // 128      # 32 n-tiles
FREE = 512         # matmul moving free-dim chunk
NCH = N // FREE    # 8 chunks
HALF = N // 2      # x loads in half-sample tiles
FP = mybir.dt.float32
BF = mybir.dt.bfloat16  # matmul operand dtype (fp32 PSUM accumulation)

# Results of the last run (for test harness introspection).
LAST_RESULTS = None


def _build(has_qkv_bias: bool, has_p_bias: bool) -> bass.Bass:
    nc = bacc.Bacc("TRN2", target_bir_lowering=False, debug=False)

    x_d = nc.declare_dram_parameter("x", [BPC, C, N], BF, isOutput=False)
    wk_d = nc.declare_dram_parameter("wkT", [C, C], BF, isOutput=False)
    wv_d = nc.declare_dram_parameter("wvT", [C, C], BF, isOutput=False)
    wp_d = nc.declare_dram_parameter("wpT", [C, C], BF, isOutput=False)
    wq_d = nc.declare_dram_parameter("wqT", [C, H16], BF, isOutput=False)
    mask_d = nc.declare_dram_parameter("maskhd", [C, H], FP, isOutput=False)
    bq_d = nc.declare_dram_parameter("bq", [1, H16], BF, isOutput=False)
    bk_d = nc.declare_dram_parameter("bk16", [H16, C], FP, isOutput=False)
    bv_d = nc.declare_dram_parameter("bv", [1, C], BF, isOutput=False)
    bp_d = nc.declare_dram_parameter("bp", [C], FP, isOutput=False)
    y_d = nc.declare_dram_parameter("y", [BPC, C, N], FP, isOutput=True)

    AF = mybir.ActivationFunctionType
    OP = mybir.AluOpType

    with tile.TileContext(nc) as tc, ExitStack() as ctx:
        consts = ctx.enter_context(tc.tile_pool(name="consts", bufs=1))
        xpool = ctx.enter_context(tc.tile_pool(name="xpool", bufs=3))
        xtpool = ctx.enter_context(tc.tile_pool(name="xtpool", bufs=1))
        apool = ctx.enter_context(tc.tile_pool(name="apool", bufs=2))
        qpool = ctx.enter_context(tc.tile_pool(name="qpool", bufs=2))
        spool = ctx.enter_context(tc.tile_pool(name="spool", bufs=2))
        small = ctx.enter_context(tc.tile_pool(name="small", bufs=2))
        wppool = ctx.enter_context(tc.tile_pool(name="wppool", bufs=2))
        opool = ctx.enter_context(tc.tile_pool(name="opool", bufs=4))
        ps_mm = ctx.enter_context(tc.tile_pool(name="ps_mm", bufs=3, space="PSUM"))
        ps_q = ctx.enter_context(tc.tile_pool(name="ps_q", bufs=3, space="PSUM"))
        ps_ctx = ctx.enter_context(tc.tile_pool(name="ps_ctx", bufs=1, space="PSUM"))

        # ---- weights into SBUF (single rearranged DMAs, on Sync/HWDGE) ----
        wk_sb = consts.tile([128, CT, C], BF)
        wv_sb = consts.tile([128, CT, C], BF)
        wp_sb = consts.tile([128, CT, C], BF)
        wq_sb = consts.tile([128, CT, H16], BF)
        mask_sb = consts.tile([128, CT, H], FP)
        bp_sb = consts.tile([128, CT], FP)
        # Weights issue on ScalarE's HWDGE queue so the Sync queue is free
        # for x loads from the first instruction. wv (needed by the first
        # V matmuls) goes first as 4 contiguous per-ct DMAs (fast), the
        # rest as single rearranged DMAs.
        for ct in range(CT):
            nc.scalar.dma_start(
                out=wv_sb[:, ct, :], in_=wv_d[ct * 128:(ct + 1) * 128, :]
            )
        nc.scalar.dma_start(
            out=wq_sb[:], in_=wq_d.rearrange("(ct p) h -> p ct h", p=128)
        )
        nc.scalar.dma_start(
            out=wk_sb[:], in_=wk_d.rearrange("(ct p) o -> p ct o", p=128)
        )
        nc.scalar.dma_start(
            out=wp_sb[:], in_=wp_d.rearrange("(ct p) o -> p ct o", p=128)
        )
        nc.scalar.dma_start(
            out=mask_sb[:], in_=mask_d.rearrange("(ct p) h -> p ct h", p=128)
        )
        nc.scalar.dma_start(out=bp_sb[:], in_=bp_d.rearrange("(o p) -> p o", p=128))

        identity = consts.tile([128, 128], FP)
        make_identity(nc, identity[:])

        if has_qkv_bias:
            bq_sb = consts.tile([1, H16], BF)
            bk_sb = consts.tile([H16, C], FP)
            bv_sb = consts.tile([1, C], BF)
            ones_row = consts.tile([1, FREE], BF)
            nc.sync.dma_start(out=bq_sb[:], in_=bq_d[:, :])
            nc.sync.dma_start(out=bk_sb[:], in_=bk_d[:, :])
            nc.sync.dma_start(out=bv_sb[:], in_=bv_d[:, :])
            nc.vector.memset(ones_row[:], 1.0)

        # Per-sample state carried between emission phases.
        st = [dict() for _ in range(BPC)]

        def emit_x_load_half(b, half, eng, widths):
            """One half-sample of x (bf16, host-converted) HBM -> SBUF."""
            xh = xpool.tile([128, CT, HALF], BF, tag="xh", name="xh")
            pos = 0
            for w in widths:
                for ct in range(CT):
                    eng.dma_start(
                        out=xh[:, ct, pos:pos + w],
                        in_=x_d[b, ct * 128:(ct + 1) * 128,
                                half * HALF + pos:half * HALF + pos + w],
                    )
                pos += w
            st[b].setdefault("xh", {})[half] = xh

        def emit_xT(b):
            """XBAR transposes x^T tiles: per (ct, half) one instruction."""
            xT = xtpool.tile([128, NT, CT, 128], BF, tag="xT", name="xT")
            for half in range(2):
                xh = st[b]["xh"][half]
                for ct in range(CT):
                    nc.sync.dma_start_transpose(
                        out=xT[:, half * (NT // 2):(half + 1) * (NT // 2), ct, :],
                        in_=xh[:, ct, :],
                    )
            st[b]["xT"] = xT

        def emit_vq(b, chunks):
            """V-GEMM blocks + qT-GEMM + evictions for the given chunks."""
            if "a" not in st[b]:
                st[b]["a"] = apool.tile([128, CT, N], BF, tag="a", name="a_sb")
                # expq in two half tiles so the first scoresT transpose only
                # depends on the first four exp chunks.
                st[b]["expq"] = [
                    qpool.tile([H16, HALF], BF, tag="expq", name="expq")
                    for _ in range(2)
                ]
                st[b]["zc"] = small.tile([H16, NCH], FP, tag="zc", name="zc")
            a_sb = st[b]["a"]
            zc = st[b]["zc"]
            for chk in chunks:
                half, lsl = chk // (NCH // 2), chk % (NCH // 2)
                xh = st[b]["xh"][half]
                expq = st[b]["expq"][half]
                csl = slice(chk * FREE, (chk + 1) * FREE)
                lcsl = slice(lsl * FREE, (lsl + 1) * FREE)
                # q first: its exp gates the scoresT transpose -> xs phase,
                # so get it off the PE before the chunk's V matmuls.
                q_ps = ps_q.tile(
                    [H16, FREE], FP, tag="q2k", name="q_ps",
                    padded_shape=[128, FREE],
                )
                for ct in range(CT):
                    last = (ct == CT - 1) and not has_qkv_bias
                    nc.tensor.matmul(
                        q_ps[:], wq_sb[:, ct, :], xh[:, ct, lcsl],
                        start=(ct == 0), stop=last,
                    )
                if has_qkv_bias:
                    nc.tensor.matmul(
                        q_ps[:], bq_sb[:], ones_row[:], start=False, stop=True
                    )
                # exp + per-chunk Z accumulation in one ScalarE op.
                nc.scalar.activation(
                    out=expq[:, lcsl], in_=q_ps[:], func=AF.Exp,
                    accum_out=zc[:, chk:chk + 1],
                )
                for i in range(CT):
                    v_ps = ps_mm.tile([128, FREE], FP, tag="mm512", name="v_ps")
                    for ct in range(CT):
                        last = (ct == CT - 1) and not has_qkv_bias
                        nc.tensor.matmul(
                            v_ps[:],
                            wv_sb[:, ct, i * 128:(i + 1) * 128],
                            xh[:, ct, lcsl],
                            start=(ct == 0), stop=last,
                        )
                    if has_qkv_bias:
                        nc.tensor.matmul(
                            v_ps[:], bv_sb[:, i * 128:(i + 1) * 128],
                            ones_row[:], start=False, stop=True,
                        )
                    # Pure relu eviction (ctx folded into Wp'):
                    # alternate ScalarE/DVE.
                    if (chk + i) % 2 == 0:
                        nc.scalar.activation(
                            out=a_sb[:, i, csl], in_=v_ps[:], func=AF.Relu
                        )
                    else:
                        nc.vector.tensor_scalar_max(
                            out=a_sb[:, i, csl], in0=v_ps[:], scalar1=0.0
                        )
        def emit_scoresT(b, half):
            """exp'd scores to [n,16] orientation via XBAR. All XBAR
            transposes stay on the Sync queue: the crossbar is a shared
            resource and concurrent transposes from two queues corrupt
            each other."""
            scoresT = spool.tile(
                [128, NT // 2, H16], BF, tag="scoresT", name="scoresT"
            )
            nc.sync.dma_start_transpose(
                out=scoresT[:], in_=st[b]["expq"][half][:]
            )
            st[b].setdefault("scoresT", {})[half] = scoresT

        def emit_xs(b):
            """xs[h,c] = sum_n exp(q)[n,h] x[c,n]."""
            xT = st[b]["xT"]
            xs_ps = ps_ctx.tile(
                [H16, C], FP, tag="ctx2k", name="xs_ps", padded_shape=[128, C]
            )
            for nt in range(NT):
                scoresT = st[b]["scoresT"][nt // (NT // 2)]
                nc.tensor.matmul(
                    xs_ps[:], scoresT[:, nt % (NT // 2), :], xT[:, nt, :, :],
                    start=(nt == 0), stop=(nt == NT - 1),
                )
            st[b]["xs_ps"] = xs_ps

        def emit_fin(b):
            """ctx = (xs @ WkT)/Z -> ctxv[c] -> Wp' = Wp * ctxv."""
            xs_ps = st[b]["xs_ps"]
            xs_sb = small.tile([H16, C], BF, tag="xs_sb", name="xs_sb")
            nc.vector.tensor_copy(out=xs_sb[:], in_=xs_ps[:])
            xsT = small.tile([128, CT, H16], BF, tag="xsT", name="xsT")
            nc.sync.dma_start_transpose(out=xsT[:], in_=xs_sb[:])
            ctxf_ps = ps_q.tile(
                [H16, C], FP, tag="q2k", name="ctxf_ps", padded_shape=[128, C]
            )
            for ct in range(CT):
                nc.tensor.matmul(
                    ctxf_ps[:], xsT[:, ct, :], wk_sb[:, ct, :],
                    start=(ct == 0), stop=(ct == CT - 1),
                )
            zsum = small.tile([H16, 1], FP, tag="zsum", name="zsum")
            nc.vector.reduce_sum(
                out=zsum[:], in_=st[b]["zc"][:], axis=mybir.AxisListType.X
            )
            invz = small.tile([H16, 1], FP, tag="invz", name="invz")
            nc.vector.reciprocal(out=invz[:], in_=zsum[:])
            ctxn = small.tile([H16, C], FP, tag="ctxn", name="ctxn")
            nc.vector.tensor_scalar_mul(out=ctxn[:], in0=ctxf_ps[:], scalar1=invz[:])
            if has_qkv_bias:
                ctxb = small.tile([H16, C], FP, tag="ctxb", name="ctxb")
                nc.vector.tensor_tensor(
                    out=ctxb[:], in0=ctxn[:], in1=bk_sb[:], op=OP.add
                )
                ctxn = ctxb
            ctxv = small.tile([128, CT], FP, tag="ctxv", name="ctxv")
            for i in range(CT):
                ctxnT_ps = ps_q.tile(
                    [128, H16], FP, tag="q2k", name="ctxnT_ps",
                    padded_shape=[128, C],
                )
                nc.tensor.transpose(
                    ctxnT_ps[:], ctxn[:, i * 128:(i + 1) * 128],
                    identity[:H16, :H16],
                )
                junk = small.tile([128, H], FP, tag="junk", name="junk")
                nc.vector.tensor_tensor(
                    out=junk[:], in0=ctxnT_ps[:, :H], in1=mask_sb[:, i, :],
                    op=OP.mult,
                )
                nc.vector.reduce_sum(
                    out=ctxv[:, i:i + 1], in_=junk[:], axis=mybir.AxisListType.X
                )
            wp2 = wppool.tile([128, CT, C], BF, tag="wp2", name="wp2")
            for ct in range(CT):
                nc.vector.tensor_scalar_mul(
                    out=wp2[:, ct, :], in0=wp_sb[:, ct, :],
                    scalar1=ctxv[:, ct:ct + 1],
                )
            st[b]["wp2"] = wp2

        def emit_p(b, o_range=None):
            """Output projection with folded weights + bias + store."""
            a_sb = st[b]["a"]
            wp2 = st[b]["wp2"]
            QS = 1024  # output staging quarter-rows
            for o in (o_range if o_range is not None else range(CT)):
                for q4 in range(4):
                    o_sb = opool.tile([128, QS], FP, tag="osb", name="o_sb")
                    for lc in range(2):
                        chk = q4 * 2 + lc
                        p_ps = ps_mm.tile([128, FREE], FP, tag="mm512", name="p_ps")
                        csl = slice(chk * FREE, (chk + 1) * FREE)
                        for c2 in range(CT):
                            nc.tensor.matmul(
                                p_ps[:],
                                wp2[:, c2, o * 128:(o + 1) * 128],
                                a_sb[:, c2, csl],
                                start=(c2 == 0), stop=(c2 == CT - 1),
                            )
                        osl = slice(lc * FREE, (lc + 1) * FREE)
                        if has_p_bias:
                            if chk % 2 == 0:
                                nc.vector.tensor_scalar_add(
                                    out=o_sb[:, osl], in0=p_ps[:],
                                    scalar1=bp_sb[:, o:o + 1],
                                )
                            else:
                                nc.scalar.add(
                                    o_sb[:, osl], p_ps[:], add=bp_sb[:, o:o + 1]
                                )
                        else:
                            if chk % 2 == 0:
                                nc.vector.tensor_copy(out=o_sb[:, osl], in_=p_ps[:])
                            else:
                                nc.scalar.copy(o_sb[:, osl], p_ps[:])
                    # y stores alternate between the ScalarE and Sync HWDGE
                    # queues: two queues keep more store DMAs in flight,
                    # which shortens the end-of-kernel drain.
                    eng = nc.scalar if (o * 4 + q4) % 2 == 0 else nc.sync
                    eng.dma_start(
                        out=y_d[b, o * 128:(o + 1) * 128,
                                q4 * QS:(q4 + 1) * QS],
                        in_=o_sb[:],
                    )

        # ---- emission schedule (PE queue order == emission order) ----
        # Queue discipline: all XBAR transposes serialize on Sync (the
        # crossbar is shared; DMA_TRANSPOSE also occupies its sequencer for
        # the whole transfer). x loads that would queue behind transposes go
        # on Scalar instead. b1's h1 load is emitted after b0's x readers so
        # the xpool slot-reuse dependencies cover all of b0's accesses.
        # b1 V/q chunks are emitted around b0's finalize and P phases so the
        # in-order PE queue never waits on the ctx-finalize DMA/DVE chains.
        emit_x_load_half(0, 0, nc.sync, [512, 512, 1024])
        emit_x_load_half(0, 1, nc.sync, [2048])
        emit_vq(0, range(NCH))
        emit_xT(0)
        emit_scoresT(0, 0)
        emit_x_load_half(1, 0, nc.sync, [2048])
        emit_x_load_half(1, 1, nc.sync, [2048])
        emit_scoresT(0, 1)
        emit_xs(0)
        emit_vq(1, range(0, 2))
        emit_fin(0)
        emit_vq(1, range(2, NCH))
        emit_xT(1)
        emit_scoresT(1, 0)
        emit_scoresT(1, 1)
        emit_p(0, range(0, 3))
        emit_xs(1)
        emit_fin(1)
        emit_p(0, range(3, CT))  # hides b1's ctx-finalize latency
        emit_p(1)

    nc.compile()
    return nc


_NC_CACHE = {}


def kernel(x, Wqkv, bqkv, Wp, bp):
    global LAST_RESULTS
    bf16 = ml_dtypes.bfloat16
    # x is consumed by the device in bf16 (fp32 streams through the PE at
    # half rate); converting on host halves the x HBM traffic as well.
    x = np.ascontiguousarray(np.asarray(x, dtype=np.float32).astype(bf16))
    Wqkv = np.asarray(Wqkv, dtype=np.float32)
    bqkv = np.asarray(bqkv, dtype=np.float32)
    Wp = np.asarray(Wp, dtype=np.float32)
    bp = np.asarray(bp, dtype=np.float32)

    # Host-side weight layout prep (tiny, one-time).
    r = Wqkv.reshape(H, 1 + 2 * HD, C)
    wqT = np.zeros((C, H16), dtype=np.float32)
    wqT[:, :H] = r[:, 0, :].T
    wqT = wqT.astype(bf16)
    wkT = np.ascontiguousarray(r[:, 1:1 + HD, :].reshape(C, C).T).astype(bf16)
    wvT = np.ascontiguousarray(r[:, 1 + HD:, :].reshape(C, C).T).astype(bf16)
    wpT = np.ascontiguousarray(Wp.T).astype(bf16)                     # [C, o]
    rb = bqkv.reshape(H, 1 + 2 * HD)
    bq = np.zeros((1, H16), dtype=np.float32)
    bq[0, :H] = rb[:, 0]
    bq = bq.astype(bf16)
    bk16 = np.tile(
        rb[:, 1:1 + HD].reshape(1, C), (H16, 1)
    ).astype(np.float32)
    bv = np.ascontiguousarray(rb[:, 1 + HD:].reshape(1, C)).astype(bf16)
    maskhd = np.zeros((C, H), dtype=np.float32)
    for ch in range(C):
        maskhd[ch, ch // HD] = 1.0

    has_qkv_bias = bool(np.any(bqkv != 0.0))
    has_p_bias = bool(np.any(bp != 0.0))

    key = (has_qkv_bias, has_p_bias)
    if key not in _NC_CACHE:
        _NC_CACHE[key] = _build(*key)
    nc = _NC_CACHE[key]

    shared = {
        "wkT": wkT, "wvT": wvT, "wpT": wpT, "wqT": wqT,
        "maskhd": maskhd, "bq": bq, "bk16": bk16, "bv": bv, "bp": bp,
    }
    in_maps = [
        {"x": x[i * BPC:(i + 1) * BPC], **shared} for i in range(NCORES)
    ]
    LAST_RESULTS = run_bass_kernel_spmd(nc, in_maps, list(range(NCORES)))
    out = np.concatenate(
        [LAST_RESULTS.results[i]["y"] for i in range(NCORES)], axis=0
    )
    return out.astype(np.float32)


if __name__ == "__main__":
    rng = np.random.default_rng(0)
    x = rng.standard_normal((B, C, N), dtype=np.float32)
    Wqkv = (rng.standard_normal((H * (1 + 2 * HD), C), dtype=np.float32) * 0.02)
    bqkv = np.zeros((H * (1 + 2 * HD),), np.float32)
    Wp = rng.standard_normal((C, C), dtype=np.float32) * 0.02
    bp = np.zeros((C,), np.float32)
    y = kernel(x, Wqkv, bqkv, Wp, bp)
    print("out", y.shape, y.dtype)
